# revision 14
# baseline (speedup 1.0000x reference)
"""Trainium2 Bass kernel for nn_CausalSelfAttention (GQA + RoPE + qk-RMSNorm).

Sharding: batch x head-quad over 8 NeuronCores.
  - Core c: batch = c // 4, quad = c % 4.
  - Each core owns 4 of the 16 q heads (4*quad .. 4*quad+3) and the matching
    2 of 8 kv heads (2*quad, 2*quad+1) for ONE batch element.
  - Per core: QKV projection for its 1024 rows of w_attn over its batch's
    2048 tokens, RoPE + qk RMS norm, causal attention, partial output
    projection through its 512 columns of w_proj.
  - Host sums the 4 partial outputs per batch (no on-device collectives).

v3: the attention inner loop is scalar-engine(exp)-bound (~660ns/j-unit on
ACT vs ~460ns on PE), so PE work from the NEXT group's QKV projection and
the PREVIOUS group's out-projection is woven at j-unit granularity between
attention matmuls (engine queues are in-order; emission order is the
schedule).  Other v3 properties:
  - single activation-table set (natural_log_exp_and_others): rms-norm
    rsqrt computed as exp(-0.5*ln(ms)), explicit table preload, so the
    compiler never inserts another table load.
  - qkv post-processing is scalar-engine only (copy+square+ln+exp); the
    norm scale multiply is folded into the rope muls on DVE.
  - softmax denominator: DVE-incremental bf16 accumulation of the exp'd
    tiles into 2 alternating accumulators, then a 2-tile ones-matmul.
  - PSUM: mm2 + s2 + y2 + d1 + op1 = 8 banks; out-projection has its own
    bank so woven op matmuls never wait on live attention banks.

Matmuls run in bf16 with fp32 PSUM accumulation; softmax/statistics fp32.
Self-contained: hardcodes all shapes from the problem spec.
"""

import math
import numpy as np
import ml_dtypes
from contextlib import ExitStack

# ---- problem constants (hardcoded per spec) ----
B, T, C = 2, 2048, 2048
N_HEAD, N_KV_HEAD, HD = 16, 8, 128
KV_DIM = N_KV_HEAD * HD
EPS = 1.1920929e-07
N_CORES = 8
P = 128
TG = 512                                 # token group (matmul N)
G = T // TG                              # 4 token groups per core
KT = C // P                              # 16 contraction tiles
QH = 4                                   # q heads per core
KVH = 2                                  # kv heads per core
MQ = QH + 2 * KVH                        # 8 row-quarters of the 1024 QKV rows
NJ = T // P                              # 16 k tiles
SCALE = 1.0 / math.sqrt(HD)

BF16 = ml_dtypes.bfloat16

_CACHE = {}


class Weaver:
    """Ordered filler stream of ~213ns-granular PE-op closures.

    take(n) emits the next n closures; insert_later(fn, d) schedules fn
    after d more closures have been emitted (for cross-engine-dependent
    ops like the ssq matmul that must trail the ACT square).
    """

    def __init__(self):
        self.ops = []
        self.idx = 0

    def add(self, fn):
        self.ops.append(fn)

    def insert_later(self, fn, delay):
        self.ops.insert(min(self.idx + delay, len(self.ops)), fn)

    def remaining(self):
        return len(self.ops) - self.idx

    def take(self, n):
        stop = min(self.idx + n, len(self.ops))
        while self.idx < stop:
            fn = self.ops[self.idx]
            self.idx += 1
            fn()
            stop = min(stop, len(self.ops))

    def drain(self):
        while self.idx < len(self.ops):
            fn = self.ops[self.idx]
            self.idx += 1
            fn()


# --------------------------------------------------------------------------
# device program
# --------------------------------------------------------------------------

def _emit(tc, out_ap, t_in):
    import concourse.bass as bass  # noqa: F401
    import concourse.mybir as mybir

    f32 = mybir.dt.float32
    bf16 = mybir.dt.bfloat16
    AF = mybir.ActivationFunctionType
    nc = tc.nc

    x_d = t_in["x_sw"]
    wq_d = t_in["wq_sw"]
    wp_d = t_in["wp_sw"]
    cs_d = t_in["cs_sw"]
    trineg_d = t_in["trineg_sw"]
    eye_d = t_in["eye_sw"]

    # preload the one activation table set that covers every function this
    # kernel uses (exp/ln/square/copy) so the compiler's table-load pass
    # never needs to thrash between per-function sets
    nc.scalar.add_instruction(
        mybir.InstLoadActFuncSet(
            name=nc.scalar.bass.get_next_instruction_name(),
            act_func_set_id=6,   # natural_log_exp_and_others
            ins=[],
            outs=[],
        )
    )

    with ExitStack() as root:
        const = root.enter_context(tc.tile_pool(name="const", bufs=1))
        xin = root.enter_context(tc.tile_pool(name="xin", bufs=2))
        # fine-grained interleaved staging over 3 DMA queues: wq-front on
        # sync, x on gpsimd, wq-back + consts on scalar.
        wq_sb = const.tile([P, KT, MQ * P], bf16)
        x0_sb = xin.tile([P, KT, TG], bf16, tag="xb")
        for k0 in range(0, KT, 2):
            # group-0 batch-1 consumes m=(4,5,6,7): k-heads + v
            nc.sync.dma_start(out=wq_sb[:, k0:k0 + 2, 4 * P:],
                              in_=wq_d[:, k0:k0 + 2, 4 * P:])
            nc.gpsimd.dma_start(out=x0_sb[:, k0:k0 + 2, :],
                                in_=x_d[:, 0, k0:k0 + 2, :])
        for k0 in range(0, KT, 4):
            nc.scalar.dma_start(out=wq_sb[:, k0:k0 + 4, 0:4 * P],
                                in_=wq_d[:, k0:k0 + 4, 0:4 * P])
        eye_sb = const.tile([P, P], bf16)
        nc.scalar.dma_start(out=eye_sb[:], in_=eye_d)
        cs_sb = const.tile([P, 2, T], bf16)
        nc.scalar.dma_start(out=cs_sb[:], in_=cs_d)
        trineg_sb = const.tile([P, P], bf16)
        nc.scalar.dma_start(out=trineg_sb[:], in_=trineg_d)
        wp_sb = const.tile([P, QH, C], bf16)
        nc.scalar.dma_start(out=wp_sb[:], in_=wp_d)
        eps_sb = const.tile([P, 1], f32)
        nc.vector.memset(eps_sb[:], EPS)
        onesm_sb = const.tile([P, P], bf16)
        nc.vector.memset(onesm_sb[:], 1.0)

        big = root.enter_context(tc.tile_pool(name="big", bufs=1))
        qn = [big.tile([P, T], bf16, name=f"qn{m}", tag=f"qn{m}")
              for m in range(6)]
        vT_sb = big.tile([P, KVH, NJ, P], bf16, tag="vT")  # [ktok, vh, j, d]
        yT = [big.tile([P, T], bf16, name=f"yT{h}", tag=f"yT{h}")
              for h in range(QH)]

        # PSUM: 8 banks total
        mm_ps = root.enter_context(tc.tile_pool(name="mmps", bufs=2, space="PSUM"))
        s_ps = root.enter_context(tc.tile_pool(name="sps", bufs=2, space="PSUM"))
        y_ps = root.enter_context(tc.tile_pool(name="yps", bufs=2, space="PSUM"))
        d_ps = root.enter_context(tc.tile_pool(name="dps", bufs=1, space="PSUM"))
        o_ps = root.enter_context(tc.tile_pool(name="ops", bufs=1, space="PSUM"))

        sqp = root.enter_context(tc.tile_pool(name="sq", bufs=3))
        rnp = root.enter_context(tc.tile_pool(name="rn", bufs=3))
        ptp = root.enter_context(tc.tile_pool(name="pt", bufs=8))
        accp = root.enter_context(tc.tile_pool(name="acc", bufs=4))
        denp = root.enter_context(tc.tile_pool(name="den", bufs=2))
        vtmp = root.enter_context(tc.tile_pool(name="vtmp", bufs=2))
        xswp = root.enter_context(tc.tile_pool(name="xswp", bufs=6))
        ropet = root.enter_context(tc.tile_pool(name="ropet", bufs=3))
        ostg = root.enter_context(tc.tile_pool(name="ost", bufs=2))

        rns = {}     # (m, g) -> norm-scale tile
        xsws = {}    # (m, g) -> half-swapped copy for rope
        xbs = {0: x0_sb}

        def prefetch_x(g):
            if g >= G or g in xbs:
                return
            xb = xin.tile([P, KT, TG], bf16, tag="xb", name="xb")
            nc.sync.dma_start(out=xb[:, 0:8, :], in_=x_d[:, g, 0:8, :])
            nc.gpsimd.dma_start(out=xb[:, 8:16, :], in_=x_d[:, g, 8:16, :])
            xbs[g] = xb

        def post_qk(m, g, ps, mi, W=None):
            """Scalar-engine-only post for a q/k row-quarter; the ssq matmul
            (PE, gated on the ACT square) is deferred via the weaver.
            Returns the deferred closure when W is None."""
            gsl = slice(g * TG, (g + 1) * TG)
            nc.scalar.copy(qn[m][:, gsl], ps[:])
            sq = sqp.tile([P, TG], bf16)
            nc.scalar.activation(sq[:], ps[:], AF.Square)
            xsw = xswp.tile([P, TG], bf16, tag="xsw")
            eng = nc.gpsimd if mi % 2 == 0 else nc.sync
            eng.dma_start(out=xsw[0:64, :], in_=qn[m][64:128, gsl])
            eng.dma_start(out=xsw[64:128, :], in_=qn[m][0:64, gsl])
            xsws[(m, g)] = xsw

            def late():
                ssq = d_ps.tile([P, TG], f32, tag="d")
                nc.tensor.matmul(ssq[:], onesm_sb[:], sq[:],
                                 start=True, stop=True)
                lnv = rnp.tile([P, TG], f32, tag="ln")
                nc.scalar.activation(lnv[:], ssq[:], AF.Ln,
                                     bias=eps_sb[:], scale=1.0 / HD)
                rn = rnp.tile([P, TG], bf16, tag="rn")
                nc.scalar.activation(rn[:], lnv[:], AF.Exp, scale=-0.5)
                rns[(m, g)] = rn

            if W is None:
                return late
            W.insert_later(late, 6)
            return None

        def post_v(vh, g, ps, W=None):
            """Transpose the v quarter into [ktok, d] layout."""
            vtm = vtmp.tile([P, TG], bf16)
            nc.vector.tensor_copy(vtm[:], ps[:])

            def mk_tp(jj):
                def f():
                    tp = s_ps.tile([P, P], bf16, tag="s")
                    nc.tensor.transpose(
                        tp[:], vtm[:, jj * P:(jj + 1) * P], eye_sb[:])
                    nc.vector.tensor_copy(vT_sb[:, vh, 4 * g + jj], tp[:])
                return f

            for jj in range(4):
                if W is None:
                    mk_tp(jj)()
                else:
                    W.insert_later(mk_tp(jj), 2 + jj)

        def emit_rope(g):
            """Rope + fused rms-norm scale for group g (k quarters first)."""
            gsl = slice(g * TG, (g + 1) * TG)
            for m in (4, 5, 0, 1, 2, 3):
                xsw = xsws.pop((m, g))
                rn = rns.pop((m, g))
                t1 = ropet.tile([P, TG], bf16, tag="t1")
                nc.vector.tensor_mul(t1[:], qn[m][:, gsl], cs_sb[:, 0, gsl])
                nc.vector.tensor_mul(xsw[:], xsw[:], cs_sb[:, 1, gsl])
                nc.vector.tensor_add(t1[:], t1[:], xsw[:])
                nc.vector.tensor_mul(qn[m][:, gsl], t1[:], rn[:])

        def emit_qkv0():
            """Group-0 QKV: k-outer over two 4-chain batches on disjoint
            PSUM banks so batch-1's post never stalls batch-2's matmuls.
            The d/o-bank ssq matmuls (late closures) run only after the
            v-chain banks have been drained by their DVE copies."""
            xb = xbs[0]
            # batch 1: k-heads + v on [mm, mm, y, y]; their post-work (vtm
            # copies, transposes, ssq) lands on then-idle s/d rings
            pss1 = [mm_ps.tile([P, TG], f32, tag="mm", name="b1a"),
                    mm_ps.tile([P, TG], f32, tag="mm", name="b1b"),
                    y_ps.tile([P, TG], f32, tag="y", name="b1c"),
                    y_ps.tile([P, TG], f32, tag="y", name="b1d")]
            batch1 = (4, 5, 6, 7)
            for k in range(KT):
                for i, m in enumerate(batch1):
                    nc.tensor.matmul(
                        pss1[i][:],
                        wq_sb[:, k, m * P:(m + 1) * P],
                        xb[:, k],
                        start=(k == 0),
                        stop=(k == KT - 1),
                    )
            lates = [post_qk(4, 0, pss1[0], 0), post_qk(5, 0, pss1[1], 1)]
            post_v(0, 0, pss1[2])
            post_v(1, 0, pss1[3])
            for late in lates:
                late()
            # batch 2: the 4 q-heads on [s, s, d, op]
            pss2 = [s_ps.tile([P, TG], f32, tag="s", name="b2a"),
                    s_ps.tile([P, TG], f32, tag="s", name="b2b"),
                    d_ps.tile([P, TG], f32, tag="d", name="b2c"),
                    o_ps.tile([P, TG], f32, tag="op", name="b2d")]
            batch2 = (0, 1, 2, 3)
            for k in range(KT):
                for i, m in enumerate(batch2):
                    nc.tensor.matmul(
                        pss2[i][:],
                        wq_sb[:, k, m * P:(m + 1) * P],
                        xb[:, k],
                        start=(k == 0),
                        stop=(k == KT - 1),
                    )
            lates = []
            for i, m in enumerate(batch2):
                lates.append(post_qk(m, 0, pss2[i], i))
            for late in lates:
                late()
            emit_rope(0)

        def build_filler(W, g_qkv, g_op):
            """QKV chains for group g_qkv + out-projection for group g_op,
            chopped into ~4-matmul closures."""
            if g_qkv < G:
                xb = xbs[g_qkv]
                for mi, m in enumerate((4, 5, 0, 1, 2, 3, 6, 7)):
                    ps_box = [None]

                    def mk_q(m, qtr, mi, ps_box):
                        def f():
                            if qtr == 0:
                                ps_box[0] = mm_ps.tile([P, TG], f32, tag="mm", name="qkvchain")
                            for k in range(4 * qtr, 4 * qtr + 4):
                                nc.tensor.matmul(
                                    ps_box[0][:],
                                    wq_sb[:, k, m * P:(m + 1) * P],
                                    xb[:, k],
                                    start=(k == 0),
                                    stop=(k == KT - 1),
                                )
                            if qtr == 3:
                                if m < 6:
                                    post_qk(m, g_qkv, ps_box[0], mi, W)
                                else:
                                    post_v(m - 6, g_qkv, ps_box[0], W)
                        return f

                    for qtr in range(4):
                        W.add(mk_q(m, qtr, mi, ps_box))
            if g_op is not None and g_op >= 0:
                for tt in range(4 * g_op, 4 * g_op + 4):
                    ost_box = [None]

                    def mk_o(tt, og, ost_box):
                        def f():
                            if og == 0:
                                ost_box[0] = ostg.tile([P, C], bf16, name="ostw")
                            ost = ost_box[0]
                            op = o_ps.tile([P, TG], f32, tag="op")
                            for h in range(QH):
                                nc.tensor.matmul(
                                    op[:], yT[h][:, tt * P:(tt + 1) * P],
                                    wp_sb[:, h, og * TG:(og + 1) * TG],
                                    start=(h == 0), stop=(h == QH - 1),
                                )
                            nc.vector.tensor_copy(
                                ost[:, og * TG:(og + 1) * TG], op[:])
                            eng = nc.sync if og % 2 == 0 else nc.gpsimd
                            eng.dma_start(
                                out=out_ap[tt * P:(tt + 1) * P,
                                           og * TG:(og + 1) * TG],
                                in_=ost[:, og * TG:(og + 1) * TG])
                        return f

                    for og in range(4):
                        W.add(mk_o(tt, og, ost_box))

        def emit_attn_woven(g, W):
            """Attention for group g with filler woven per j-unit."""
            gsl = slice(g * TG, (g + 1) * TG)
            jn = 4 * g + 4
            units_left = QH * jn
            for qh in range(QH):
                kv = qh // 2
                k_t = qn[4 + kv]
                q_g = qn[qh][:, gsl]
                yp = y_ps.tile([P, TG], f32, tag="y")
                acc = [None, None]
                prev = None       # (j, pt, off)
                for j in range(jn):
                    off = (j - 4 * g) * P if j >= 4 * g else 0
                    diag = j >= 4 * g
                    s = s_ps.tile([P, TG], f32, tag="s")
                    nc.tensor.matmul(
                        s[:, off:],
                        k_t[:, j * P:(j + 1) * P],
                        q_g[:, off:],
                        start=True,
                        stop=not diag,
                        skip_group_check=diag,
                    )
                    if diag:
                        nc.tensor.matmul(
                            s[:, off:off + P], trineg_sb[:], eye_sb[:],
                            start=False, stop=True,
                            skip_group_check=True,
                        )
                    pt = ptp.tile([P, TG], bf16)
                    nc.scalar.activation(pt[:, off:], s[:, off:], AF.Exp,
                                         scale=SCALE)
                    a = 0 if (off > 0) else (j % 2)
                    if acc[a] is None:
                        assert off == 0
                        acc[a] = accp.tile([P, TG], bf16, tag="acc",
                                           name=f"acc{a}")
                        nc.vector.tensor_copy(acc[a][:], pt[:])
                    else:
                        nc.vector.tensor_add(acc[a][:, off:],
                                             acc[a][:, off:], pt[:, off:])
                    # filler between this unit's score and last unit's PV
                    nfill = 2
                    if units_left > 0:
                        nfill = max(2, min(6, -(-W.remaining() // units_left)))
                    W.take(nfill)
                    units_left -= 1
                    if prev is not None:
                        pj, ppt, poff = prev
                        nc.tensor.matmul(
                            yp[:, poff:], vT_sb[:, kv, pj], ppt[:, poff:],
                            start=(pj == 0), stop=False,
                            skip_group_check=True,
                        )
                    prev = (j, pt, off)
                pj, ppt, poff = prev
                nc.tensor.matmul(
                    yp[:, poff:], vT_sb[:, kv, pj], ppt[:, poff:],
                    start=(pj == 0), stop=True,
                    skip_group_check=True,
                )
                W.take(2)
                # d-bank allocated only now so woven ssq matmuls never wait
                # behind a head-long hold
                dp = d_ps.tile([P, TG], f32, tag="d")
                na = 2 if acc[1] is not None else 1
                nc.tensor.matmul(dp[:], onesm_sb[:], acc[0][:],
                                 start=True, stop=(na == 1),
                                 skip_group_check=True)
                if na == 2:
                    nc.tensor.matmul(dp[:], onesm_sb[:], acc[1][:],
                                     start=False, stop=True,
                                     skip_group_check=True)
                den = denp.tile([P, TG], f32)
                if qh == QH - 1:
                    for u in range(4):
                        usl = slice(u * P, (u + 1) * P)
                        nc.vector.reciprocal_approx_fast(den[:, usl],
                                                         dp[:, usl])
                        nc.vector.tensor_mul(
                            yT[qh][:, g * TG + u * P: g * TG + (u + 1) * P],
                            yp[:, usl], den[:, usl])
                else:
                    nc.vector.reciprocal_approx_fast(den[:], dp[:])
                    nc.vector.tensor_mul(yT[qh][:, gsl], yp[:], den[:])

        def emit_outproj_tail(g):
            for tt in range(4 * g, 4 * g + 4):
                ost = ostg.tile([P, C], bf16)
                for og in range(4):
                    op = o_ps.tile([P, TG], f32, tag="op")
                    for h in range(QH):
                        nc.tensor.matmul(
                            op[:], yT[h][:, tt * P:(tt + 1) * P],
                            wp_sb[:, h, og * TG:(og + 1) * TG],
                            start=(h == 0), stop=(h == QH - 1),
                        )
                    if og % 2 == 0:
                        nc.vector.tensor_copy(ost[:, og * TG:(og + 1) * TG],
                                              op[:])
                    else:
                        nc.scalar.copy(ost[:, og * TG:(og + 1) * TG], op[:])
                    eng = nc.sync if og % 2 == 0 else nc.gpsimd
                    eng.dma_start(
                        out=out_ap[tt * P:(tt + 1) * P,
                                   og * TG:(og + 1) * TG],
                        in_=ost[:, og * TG:(og + 1) * TG])

        # ---- schedule ----
        prefetch_x(1)
        emit_qkv0()
        for i in range(1, G + 1):
            prefetch_x(i + 1)
            W = Weaver()
            build_filler(W, g_qkv=i, g_op=i - 2)
            emit_attn_woven(i - 1, W)
            W.drain()
            if i < G:
                emit_rope(i)
        emit_outproj_tail(G - 1)


def build_nc():
    """Build and compile the (single, shared across cores) Bass program."""
    if "nc" in _CACHE:
        return _CACHE["nc"]
    import concourse.mybir as mybir
    import concourse.tile as tile
    from concourse import bacc

    bf16 = mybir.dt.bfloat16

    nc = bacc.Bacc("TRN2", target_bir_lowering=False, debug=False)
    shapes = {
        "x_sw": ((P, G, KT, TG), bf16),
        "wq_sw": ((P, KT, MQ * P), bf16),
        "wp_sw": ((P, QH, C), bf16),
        "cs_sw": ((P, 2, T), bf16),
        "trineg_sw": ((P, P), bf16),
        "eye_sw": ((P, P), bf16),
    }
    t_in = {
        name: nc.dram_tensor(name, shape, dt, kind="ExternalInput").ap()
        for name, (shape, dt) in shapes.items()
    }
    out_ap = nc.dram_tensor("out", (T, C), bf16, kind="ExternalOutput").ap()

    with tile.TileContext(nc) as tc:
        _emit(tc, out_ap, t_in)
    nc.compile()
    _CACHE["nc"] = nc
    return nc


# --------------------------------------------------------------------------
# host-side data preparation
# --------------------------------------------------------------------------

def _swizzle_ktiles(a2d):
    """[R*128, F] -> [128, R, F] picking partition-within-tile as leading."""
    r128, f = a2d.shape
    r = r128 // P
    return np.ascontiguousarray(a2d.reshape(r, P, f).transpose(1, 0, 2))


def host_prep(x, w_attn, w_proj, cos, sin):
    x = np.asarray(x, np.float32)
    w_attn = np.asarray(w_attn, np.float32)
    w_proj = np.asarray(w_proj, np.float32)
    cos = np.asarray(cos, np.float32).reshape(T, HD // 2)
    sin = np.asarray(sin, np.float32).reshape(T, HD // 2)

    # x per batch: (T, C) -> [128, g, k, t]
    x_sws = []
    for b in range(B):
        xb = x[b].reshape(G, TG, KT, P).transpose(3, 0, 2, 1)
        x_sws.append(np.ascontiguousarray(xb).astype(BF16))

    # cos/sin duplicated across both 64-partition halves: [128, 2, T]
    c2 = np.concatenate([cos.T, cos.T], axis=0)     # (128, T)
    s2 = np.concatenate([sin.T, -sin.T], axis=0)    # sign-folded for rope add
    cs_sw = np.stack([c2, s2], axis=1).astype(BF16)  # (128, 2, T)

    col = np.arange(P)[None, :]
    row = np.arange(P)[:, None]
    # M[r,c] = 0 where causal-live (c >= r), -1e30 where masked; the device
    # adds M to the diagonal score block via lhsT = M.T (out[i,j] = lhsT[j,i])
    m_mask = np.where(col >= row, 0.0, -1e30).astype(np.float32)
    trineg_sw = np.ascontiguousarray(m_mask.T).astype(BF16)
    eye_sw = np.eye(P, dtype=np.float32).astype(BF16)

    in_maps = []
    for c in range(N_CORES):
        b, q = divmod(c, 4)
        qrows = w_attn[QH * HD * q: QH * HD * (q + 1)]
        krows = w_attn[C + KVH * HD * q: C + KVH * HD * (q + 1)]
        vrows = w_attn[C + KV_DIM + KVH * HD * q: C + KV_DIM + KVH * HD * (q + 1)]
        w_sel = np.concatenate([qrows, krows, vrows], axis=0)   # (1024, C)
        wq_sw = _swizzle_ktiles(w_sel.T).astype(BF16)           # (128, 16, 1024)

        wp_sel = w_proj[:, QH * HD * q: QH * HD * (q + 1)]      # (C, 512)
        wp_sw = _swizzle_ktiles(np.ascontiguousarray(wp_sel.T)).astype(BF16)

        in_maps.append({
            "x_sw": x_sws[b],
            "wq_sw": np.ascontiguousarray(wq_sw),
            "wp_sw": np.ascontiguousarray(wp_sw),   # (128, 4, 2048)
            "cs_sw": cs_sw,
            "trineg_sw": trineg_sw,
            "eye_sw": eye_sw,
        })
    return in_maps


def run_on_hw(in_maps, trace=False, **kwargs):
    from concourse import bass_utils

    nc = build_nc()
    return bass_utils.run_bass_kernel_spmd(
        nc, in_maps, core_ids=list(range(N_CORES)), trace=trace, **kwargs
    )


def gather(res):
    """Sum the 4 partial outputs per batch -> (B, T, C) float32."""
    out = np.zeros((B, T, C), np.float32)
    for c, r in enumerate(res.results):
        out[c // 4] += r["out"].astype(np.float32)
    return out


def kernel(x, w_attn, w_proj, cos, sin):
    in_maps = host_prep(x, w_attn, w_proj, cos, sin)
    res = run_on_hw(in_maps)
    return gather(res)


# revision 15
# speedup vs baseline: 1.0341x; 1.0341x over previous
"""Trainium2 Bass kernel for nn_CausalSelfAttention (GQA + RoPE + qk-RMSNorm).

Sharding: batch x head-quad over 8 NeuronCores.
  - Core c: batch = c // 4, quad = c % 4.
  - Each core owns 4 of the 16 q heads (4*quad .. 4*quad+3) and the matching
    2 of 8 kv heads (2*quad, 2*quad+1) for ONE batch element.
  - Per core: QKV projection for its 1024 rows of w_attn over its batch's
    2048 tokens, RoPE + qk RMS norm, causal attention, partial output
    projection through its 512 columns of w_proj.
  - Host sums the 4 partial outputs per batch (no on-device collectives).

Fused per-token-group pipeline: for each 512-token group g we run
QKV -> rope/norm -> attention (flash-style, causal-sliced) -> out-proj, so
the tensor engine always has dense matmul work while exp/softmax runs on
the scalar/vector engines.

Matmuls run in bf16 with fp32 PSUM accumulation; softmax/statistics fp32.
Self-contained: hardcodes all shapes from the problem spec.
"""

import math
import numpy as np
import ml_dtypes
from contextlib import ExitStack

# ---- problem constants (hardcoded per spec) ----
B, T, C = 2, 2048, 2048
N_HEAD, N_KV_HEAD, HD = 16, 8, 128
KV_DIM = N_KV_HEAD * HD
EPS = 1.1920929e-07
N_CORES = 8
P = 128
TG = 512                                 # token group (matmul N)
G = T // TG                              # 4 token groups per core
KT = C // P                              # 16 contraction tiles
QH = 4                                   # q heads per core
KVH = 2                                  # kv heads per core
MQ = QH + 2 * KVH                        # 8 row-quarters of the 1024 QKV rows
NJ = T // P                              # 16 k tiles
SCALE = 1.0 / math.sqrt(HD)

BF16 = ml_dtypes.bfloat16

_CACHE = {}


# --------------------------------------------------------------------------
# device program
# --------------------------------------------------------------------------

def _emit(tc, out_ap, t_in):
    import concourse.bass as bass  # noqa: F401
    import concourse.mybir as mybir

    f32 = mybir.dt.float32
    bf16 = mybir.dt.bfloat16
    AF = mybir.ActivationFunctionType
    nc = tc.nc

    x_d = t_in["x_sw"]
    wq_d = t_in["wq_sw"]
    wp_d = t_in["wp_sw"]
    cs_d = t_in["cs_sw"]
    trineg_d = t_in["trineg_sw"]
    eye_d = t_in["eye_sw"]

    # preload the one activation table set covering every function used
    # (exp/ln/square/copy) so the compiler's table-load pass never thrashes
    nc.scalar.add_instruction(
        mybir.InstLoadActFuncSet(
            name=nc.scalar.bass.get_next_instruction_name(),
            act_func_set_id=6,   # natural_log_exp_and_others
            ins=[],
            outs=[],
        )
    )

    with ExitStack() as root:
        const = root.enter_context(tc.tile_pool(name="const", bufs=1))
        xin = root.enter_context(tc.tile_pool(name="xin", bufs=2))
        # fine-grained interleaved staging: QKV(g=0) runs k-outer over the
        # q-head half of wq, so chunk k-tiles of wq/x land just ahead of use.
        # wq on the sync queue, x on the gpsimd queue -- parallel streams.
        wq_sb = const.tile([P, KT, MQ * P], bf16)
        x0_sb = xin.tile([P, KT, TG], bf16, tag="xb")
        for k0 in range(0, KT, 2):
            nc.sync.dma_start(out=wq_sb[:, k0:k0 + 2, 0:4 * P],
                              in_=wq_d[:, k0:k0 + 2, 0:4 * P])
            nc.gpsimd.dma_start(out=x0_sb[:, k0:k0 + 2, :],
                                in_=x_d[:, 0, k0:k0 + 2, :])
        for k0 in range(0, KT, 4):
            nc.scalar.dma_start(out=wq_sb[:, k0:k0 + 4, 4 * P:],
                                in_=wq_d[:, k0:k0 + 4, 4 * P:])
        eye_sb = const.tile([P, P], bf16)
        nc.scalar.dma_start(out=eye_sb[:], in_=eye_d)
        cs_sb = const.tile([P, 2, T], bf16)
        nc.scalar.dma_start(out=cs_sb[:], in_=cs_d)
        trineg_sb = const.tile([P, P], bf16)
        nc.scalar.dma_start(out=trineg_sb[:], in_=trineg_d)
        wp_sb = const.tile([P, QH, C], bf16)
        nc.scalar.dma_start(out=wp_sb[:], in_=wp_d)
        eps_sb = const.tile([P, 1], f32)
        nc.vector.memset(eps_sb[:], EPS)
        onesm_sb = const.tile([P, P], bf16)
        nc.vector.memset(onesm_sb[:], 1.0)

        big = root.enter_context(tc.tile_pool(name="big", bufs=1))
        # post-rope, post-norm q (4 heads) + k (2 heads), [d, tok] layout
        qn = [big.tile([P, T], bf16, name=f"qn{m}", tag=f"qn{m}")
              for m in range(6)]
        vT_sb = big.tile([P, KVH, NJ, P], bf16, tag="vT")  # [ktok, vh, j, d]
        yT = [big.tile([P, T], bf16, name=f"yT{h}", tag=f"yT{h}")
              for h in range(QH)]

        mm_ps = root.enter_context(tc.tile_pool(name="mmps", bufs=2, space="PSUM"))
        s_ps = root.enter_context(tc.tile_pool(name="sps", bufs=3, space="PSUM"))
        y_ps = root.enter_context(tc.tile_pool(name="yps", bufs=2, space="PSUM"))
        d_ps = root.enter_context(tc.tile_pool(name="dps", bufs=1, space="PSUM"))
        sqp = root.enter_context(tc.tile_pool(name="sq", bufs=3))
        srp = root.enter_context(tc.tile_pool(name="sr", bufs=2))
        ptp = root.enter_context(tc.tile_pool(name="pt", bufs=8))
        pap = root.enter_context(tc.tile_pool(name="pa", bufs=4))
        denp = root.enter_context(tc.tile_pool(name="den", bufs=2))
        vtmp = root.enter_context(tc.tile_pool(name="vtmp", bufs=2))
        xswp = root.enter_context(tc.tile_pool(name="xswp", bufs=6))
        ropet = root.enter_context(tc.tile_pool(name="ropet", bufs=3))
        ostg = root.enter_context(tc.tile_pool(name="ost", bufs=2))

        def emit_qkv_rope(g):
            """QKV projection + norm + rope for token group g."""
            gsl = slice(g * TG, (g + 1) * TG)
            if g == 0:
                xb = x0_sb
            else:
                xb = xin.tile([P, KT, TG], bf16, tag="xb", name="xb")
                nc.sync.dma_start(out=xb[:, 0:8, :], in_=x_d[:, g, 0:8, :])
                nc.gpsimd.dma_start(out=xb[:, 8:16, :], in_=x_d[:, g, 8:16, :])
            xsws = {}

            def qkv_post(m, ps, mi):
                if m < 6:
                    # free the PSUM slot immediately: copy to SBUF first,
                    # then the whole norm chain runs off the SBUF copy, so
                    # an ACT table switch can't back up the matmul pipeline
                    nc.vector.tensor_copy(qn[m][:, gsl], ps[:])
                    # rms-norm: broadcast sum-of-squares via all-ones MM
                    sq = sqp.tile([P, TG], bf16)
                    nc.scalar.activation(sq[:], qn[m][:, gsl], AF.Square)
                    ssq = s_ps.tile([P, TG], f32, tag="s")
                    nc.tensor.matmul(ssq[:], onesm_sb[:], sq[:],
                                     start=True, stop=True)
                    lnv = srp.tile([P, TG], f32, tag="ln", name="lnv")
                    nc.scalar.activation(lnv[:], ssq[:], AF.Ln,
                                         bias=eps_sb[:], scale=1.0 / HD)
                    srb = srp.tile([P, TG], bf16, tag="rn", name="srb")
                    nc.scalar.activation(srb[:], lnv[:], AF.Exp, scale=-0.5)
                    nc.vector.tensor_mul(qn[m][:, gsl], qn[m][:, gsl], srb[:])
                    # issue the rope half-swap immediately; consumed after
                    # the m-loop.  Alternate DMA queues to avoid serializing.
                    xsw = xswp.tile([P, TG], bf16, tag="xsw")
                    eng = nc.gpsimd if mi % 2 == 0 else nc.sync
                    eng.dma_start(out=xsw[0:64, :], in_=qn[m][64:128, gsl])
                    eng.dma_start(out=xsw[64:128, :], in_=qn[m][0:64, gsl])
                    xsws[m] = xsw
                else:
                    vh = m - 6
                    vtm = vtmp.tile([P, TG], bf16)
                    nc.vector.tensor_copy(vtm[:], ps[:])
                    for jj in range(4):
                        tp = s_ps.tile([P, P], bf16, tag="s")
                        nc.tensor.transpose(
                            tp[:], vtm[:, jj * P:(jj + 1) * P], eye_sb[:])
                        nc.vector.tensor_copy(vT_sb[:, vh, 4 * g + jj], tp[:])

            if g == 0:
                # k-outer in two 4-quarter batches (4 PSUM banks each) so the
                # PE consumes wq/x chunks as the staged DMAs land
                for batch in ((0, 1, 2, 3), (4, 5, 6, 7)):
                    pss = [mm_ps.tile([P, TG], f32, tag="mm", name="ps_a"),
                           mm_ps.tile([P, TG], f32, tag="mm", name="ps_b"),
                           y_ps.tile([P, TG], f32, tag="y", name="ps_c"),
                           d_ps.tile([P, TG], f32, tag="d", name="ps_d")]
                    for k in range(KT):
                        for i, m in enumerate(batch):
                            nc.tensor.matmul(
                                pss[i][:],
                                wq_sb[:, k, m * P:(m + 1) * P],
                                xb[:, k],
                                start=(k == 0),
                                stop=(k == KT - 1),
                            )
                    for i, m in enumerate(batch):
                        qkv_post(m, pss[i], i)
            else:
                for mi, m in enumerate((4, 5, 0, 1, 2, 3, 6, 7)):
                    ps = mm_ps.tile([P, TG], f32, tag="mm")
                    for k in range(KT):
                        nc.tensor.matmul(
                            ps[:],
                            wq_sb[:, k, m * P:(m + 1) * P],
                            xb[:, k],
                            start=(k == 0),
                            stop=(k == KT - 1),
                        )
                    qkv_post(m, ps, mi)

            return xsws

        def emit_rope(g, xsws):
            """Rope for group g (k quarters first)."""
            gsl = slice(g * TG, (g + 1) * TG)
            for m in (4, 5, 0, 1, 2, 3):
                xsw = xsws[m]
                t1 = ropet.tile([P, TG], bf16, tag="t1")
                nc.vector.tensor_mul(t1[:], qn[m][:, gsl], cs_sb[:, 0, gsl])
                nc.vector.tensor_mul(xsw[:], xsw[:], cs_sb[:, 1, gsl])
                nc.vector.tensor_add(qn[m][:, gsl], t1[:], xsw[:])

        def emit_attn(g):
            """Attention for token group g."""
            gsl = slice(g * TG, (g + 1) * TG)
            jn = 4 * g + 4
            for qh in range(QH):
                kv = qh // 2
                k_t = qn[4 + kv]
                q_g = qn[qh][:, gsl]
                yp = y_ps.tile([P, TG], f32, tag="y")
                dp = d_ps.tile([P, TG], f32, tag="d")
                pts = []   # (ap, off) pending for the den chain
                for j in range(jn):
                    off = (j - 4 * g) * P if j >= 4 * g else 0
                    diag = j >= 4 * g
                    s = s_ps.tile([P, TG], f32, tag="s")
                    nc.tensor.matmul(
                        s[:, off:],
                        k_t[:, j * P:(j + 1) * P],
                        q_g[:, off:],
                        start=True,
                        stop=not diag,
                        skip_group_check=diag,
                    )
                    if diag:
                        # add -1e30 to the below-diagonal triangle on PE, so
                        # exp maps it to 0 (no cross-engine mask dependency)
                        nc.tensor.matmul(
                            s[:, off:off + P], trineg_sb[:], eye_sb[:],
                            start=False, stop=True,
                            skip_group_check=True,
                        )
                    pt = ptp.tile([P, TG], bf16)
                    nc.scalar.activation(pt[:, off:], s[:, off:], AF.Exp,
                                         scale=SCALE)
                    nc.tensor.matmul(
                        yp[:, off:], vT_sb[:, kv, j], pt[:, off:],
                        start=(j == 0), stop=(j == jn - 1),
                        skip_group_check=True,
                    )
                    # denominator: pair-add full tiles on DVE, chain on PE
                    if off == 0 and j % 2 == 0 and j + 1 < 4 * g:
                        pts.append((pt, -1))    # -1: waiting for partner
                    elif off == 0 and pts and pts[-1][1] == -1:
                        pa = pap.tile([P, TG], bf16)
                        nc.vector.tensor_add(pa[:], pts[-1][0][:], pt[:])
                        pts[-1] = (pa, 0)
                    else:
                        pts.append((pt, off))
                nd = len(pts)
                for i, (pa, off) in enumerate(pts):
                    assert off >= 0
                    nc.tensor.matmul(
                        dp[:, off:], onesm_sb[:], pa[:, off:],
                        start=(i == 0), stop=(i == nd - 1),
                        skip_group_check=True,
                    )
                den = denp.tile([P, TG], f32)
                if qh == QH - 1:
                    # chunk recip+mul per token-tile: the last head gates the
                    # out-projection, so let its first tile finish early
                    for u in range(4):
                        usl = slice(u * P, (u + 1) * P)
                        nc.vector.reciprocal_approx_fast(den[:, usl],
                                                         dp[:, usl])
                        nc.vector.tensor_mul(
                            yT[qh][:, g * TG + u * P: g * TG + (u + 1) * P],
                            yp[:, usl], den[:, usl])
                else:
                    nc.vector.reciprocal_approx_fast(den[:], dp[:])
                    nc.vector.tensor_mul(yT[qh][:, gsl], yp[:], den[:])

        def emit_outproj(g):
            """Out-projection for token group g."""
            for tt in range(4 * g, 4 * g + 4):
                ost = ostg.tile([P, C], bf16)
                for og in range(4):
                    op = mm_ps.tile([P, TG], f32, tag="mm", name="op")
                    for h in range(QH):
                        nc.tensor.matmul(
                            op[:], yT[h][:, tt * P:(tt + 1) * P],
                            wp_sb[:, h, og * TG:(og + 1) * TG],
                            start=(h == 0), stop=(h == QH - 1),
                        )
                    if og % 2 == 0:
                        nc.vector.tensor_copy(ost[:, og * TG:(og + 1) * TG], op[:])
                    else:
                        nc.scalar.copy(ost[:, og * TG:(og + 1) * TG], op[:])
                    # store each 512-col block as its copy lands, on
                    # alternating queues so the tail drains two at a time
                    eng = nc.sync if og % 2 == 0 else nc.gpsimd
                    eng.dma_start(
                        out=out_ap[tt * P:(tt + 1) * P, og * TG:(og + 1) * TG],
                        in_=ost[:, og * TG:(og + 1) * TG])

        # software pipeline: emit QKV(g+1) BEFORE attention(g) so the
        # scheduler can fill exp-gated attention bubbles with QKV matmuls;
        # attention(g) DVE ops precede rope(g+1) in the vector FIFO (rope
        # has a full iteration of slack, attention gates the out-proj)
        for i in range(G + 1):
            if i < G:
                xsws_i = emit_qkv_rope(i)
            if i >= 1:
                emit_attn(i - 1)
            if i < G:
                emit_rope(i, xsws_i)
            if i >= 1:
                emit_outproj(i - 1)


def build_nc():
    """Build and compile the (single, shared across cores) Bass program."""
    if "nc" in _CACHE:
        return _CACHE["nc"]
    import concourse.mybir as mybir
    import concourse.tile as tile
    from concourse import bacc

    bf16 = mybir.dt.bfloat16

    nc = bacc.Bacc("TRN2", target_bir_lowering=False, debug=False)
    shapes = {
        "x_sw": ((P, G, KT, TG), bf16),
        "wq_sw": ((P, KT, MQ * P), bf16),
        "wp_sw": ((P, QH, C), bf16),
        "cs_sw": ((P, 2, T), bf16),
        "trineg_sw": ((P, P), bf16),
        "eye_sw": ((P, P), bf16),
    }
    t_in = {
        name: nc.dram_tensor(name, shape, dt, kind="ExternalInput").ap()
        for name, (shape, dt) in shapes.items()
    }
    out_ap = nc.dram_tensor("out", (T, C), bf16, kind="ExternalOutput").ap()

    with tile.TileContext(nc) as tc:
        _emit(tc, out_ap, t_in)
    nc.compile()
    _CACHE["nc"] = nc
    return nc


# --------------------------------------------------------------------------
# host-side data preparation
# --------------------------------------------------------------------------

def _swizzle_ktiles(a2d):
    """[R*128, F] -> [128, R, F] picking partition-within-tile as leading."""
    r128, f = a2d.shape
    r = r128 // P
    return np.ascontiguousarray(a2d.reshape(r, P, f).transpose(1, 0, 2))


def host_prep(x, w_attn, w_proj, cos, sin):
    x = np.asarray(x, np.float32)
    w_attn = np.asarray(w_attn, np.float32)
    w_proj = np.asarray(w_proj, np.float32)
    cos = np.asarray(cos, np.float32).reshape(T, HD // 2)
    sin = np.asarray(sin, np.float32).reshape(T, HD // 2)

    # x per batch: (T, C) -> [128, g, k, t]
    x_sws = []
    for b in range(B):
        xb = x[b].reshape(G, TG, KT, P).transpose(3, 0, 2, 1)
        x_sws.append(np.ascontiguousarray(xb).astype(BF16))

    # cos/sin duplicated across both 64-partition halves: [128, 2, T]
    c2 = np.concatenate([cos.T, cos.T], axis=0)     # (128, T)
    s2 = np.concatenate([sin.T, -sin.T], axis=0)    # sign-folded for rope add
    cs_sw = np.stack([c2, s2], axis=1).astype(BF16)  # (128, 2, T)

    col = np.arange(P)[None, :]
    row = np.arange(P)[:, None]
    # M[r,c] = 0 where causal-live (c >= r), -1e30 where masked; the device
    # adds M to the diagonal score block via lhsT = M.T (out[i,j] = lhsT[j,i])
    m_mask = np.where(col >= row, 0.0, -1e30).astype(np.float32)
    trineg_sw = np.ascontiguousarray(m_mask.T).astype(BF16)
    eye_sw = np.eye(P, dtype=np.float32).astype(BF16)

    in_maps = []
    for c in range(N_CORES):
        b, q = divmod(c, 4)
        qrows = w_attn[QH * HD * q: QH * HD * (q + 1)]
        krows = w_attn[C + KVH * HD * q: C + KVH * HD * (q + 1)]
        vrows = w_attn[C + KV_DIM + KVH * HD * q: C + KV_DIM + KVH * HD * (q + 1)]
        w_sel = np.concatenate([qrows, krows, vrows], axis=0)   # (1024, C)
        wq_sw = _swizzle_ktiles(w_sel.T).astype(BF16)           # (128, 16, 1024)

        wp_sel = w_proj[:, QH * HD * q: QH * HD * (q + 1)]      # (C, 512)
        wp_sw = _swizzle_ktiles(np.ascontiguousarray(wp_sel.T)).astype(BF16)

        in_maps.append({
            "x_sw": x_sws[b],
            "wq_sw": np.ascontiguousarray(wq_sw),
            "wp_sw": np.ascontiguousarray(wp_sw),   # (128, 4, 2048)
            "cs_sw": cs_sw,
            "trineg_sw": trineg_sw,
            "eye_sw": eye_sw,
        })
    return in_maps


def run_on_hw(in_maps, trace=False, **kwargs):
    from concourse import bass_utils

    nc = build_nc()
    return bass_utils.run_bass_kernel_spmd(
        nc, in_maps, core_ids=list(range(N_CORES)), trace=trace, **kwargs
    )


def gather(res):
    """Sum the 4 partial outputs per batch -> (B, T, C) float32."""
    out = np.zeros((B, T, C), np.float32)
    for c, r in enumerate(res.results):
        out[c // 4] += r["out"].astype(np.float32)
    return out


def kernel(x, w_attn, w_proj, cos, sin):
    in_maps = host_prep(x, w_attn, w_proj, cos, sin)
    res = run_on_hw(in_maps)
    return gather(res)



# revision 16
# speedup vs baseline: 1.0367x; 1.0025x over previous
"""Trainium2 Bass kernel for nn_CausalSelfAttention (GQA + RoPE + qk-RMSNorm).

Sharding: batch x head-quad over 8 NeuronCores.
  - Core c: batch = c // 4, quad = c % 4.
  - Each core owns 4 of the 16 q heads (4*quad .. 4*quad+3) and the matching
    2 of 8 kv heads (2*quad, 2*quad+1) for ONE batch element.
  - Per core: QKV projection for its 1024 rows of w_attn over its batch's
    2048 tokens, RoPE + qk RMS norm, causal attention, partial output
    projection through its 512 columns of w_proj.
  - Host sums the 4 partial outputs per batch (no on-device collectives).

Fused per-token-group pipeline: for each 512-token group g we run
QKV -> rope/norm -> attention (flash-style, causal-sliced) -> out-proj, so
the tensor engine always has dense matmul work while exp/softmax runs on
the scalar/vector engines.

Matmuls run in bf16 with fp32 PSUM accumulation; softmax/statistics fp32.
Self-contained: hardcodes all shapes from the problem spec.
"""

import math
import numpy as np
import ml_dtypes
from contextlib import ExitStack

# ---- problem constants (hardcoded per spec) ----
B, T, C = 2, 2048, 2048
N_HEAD, N_KV_HEAD, HD = 16, 8, 128
KV_DIM = N_KV_HEAD * HD
EPS = 1.1920929e-07
N_CORES = 8
P = 128
TG = 512                                 # token group (matmul N)
G = T // TG                              # 4 token groups per core
KT = C // P                              # 16 contraction tiles
QH = 4                                   # q heads per core
KVH = 2                                  # kv heads per core
MQ = QH + 2 * KVH                        # 8 row-quarters of the 1024 QKV rows
NJ = T // P                              # 16 k tiles
SCALE = 1.0 / math.sqrt(HD)

BF16 = ml_dtypes.bfloat16

_CACHE = {}


# --------------------------------------------------------------------------
# device program
# --------------------------------------------------------------------------

def _emit(tc, out_ap, t_in):
    import concourse.bass as bass  # noqa: F401
    import concourse.mybir as mybir

    f32 = mybir.dt.float32
    bf16 = mybir.dt.bfloat16
    AF = mybir.ActivationFunctionType
    nc = tc.nc

    x_d = t_in["x_sw"]
    wq_d = t_in["wq_sw"]
    wp_d = t_in["wp_sw"]
    cs_d = t_in["cs_sw"]
    trineg_d = t_in["trineg_sw"]
    eye_d = t_in["eye_sw"]

    with ExitStack() as root:
        const = root.enter_context(tc.tile_pool(name="const", bufs=1))
        xin = root.enter_context(tc.tile_pool(name="xin", bufs=2))
        # fine-grained interleaved staging: QKV(g=0) runs k-outer over the
        # q-head half of wq, so chunk k-tiles of wq/x land just ahead of use.
        # wq on the sync queue, x on the gpsimd queue -- parallel streams.
        wq_sb = const.tile([P, KT, MQ * P], bf16)
        x0_sb = xin.tile([P, KT, TG], bf16, tag="xb")
        for k0 in range(0, KT, 2):
            nc.sync.dma_start(out=wq_sb[:, k0:k0 + 2, 0:4 * P],
                              in_=wq_d[:, k0:k0 + 2, 0:4 * P])
            nc.gpsimd.dma_start(out=x0_sb[:, k0:k0 + 2, :],
                                in_=x_d[:, 0, k0:k0 + 2, :])
        for k0 in range(0, KT, 4):
            nc.scalar.dma_start(out=wq_sb[:, k0:k0 + 4, 4 * P:],
                                in_=wq_d[:, k0:k0 + 4, 4 * P:])
        eye_sb = const.tile([P, P], bf16)
        nc.scalar.dma_start(out=eye_sb[:], in_=eye_d)
        cs_sb = const.tile([P, 2, T], bf16)
        nc.scalar.dma_start(out=cs_sb[:], in_=cs_d)
        trineg_sb = const.tile([P, P], bf16)
        nc.scalar.dma_start(out=trineg_sb[:], in_=trineg_d)
        wp_sb = const.tile([P, QH, C], bf16)
        nc.scalar.dma_start(out=wp_sb[:], in_=wp_d)
        eps_sb = const.tile([P, 1], f32)
        nc.vector.memset(eps_sb[:], EPS)
        onesm_sb = const.tile([P, P], bf16)
        nc.vector.memset(onesm_sb[:], 1.0)

        big = root.enter_context(tc.tile_pool(name="big", bufs=1))
        # post-rope, post-norm q (4 heads) + k (2 heads), [d, tok] layout
        qn = [big.tile([P, T], bf16, name=f"qn{m}", tag=f"qn{m}")
              for m in range(6)]
        vT_sb = big.tile([P, KVH, NJ, P], bf16, tag="vT")  # [ktok, vh, j, d]
        yT = [big.tile([P, T], bf16, name=f"yT{h}", tag=f"yT{h}")
              for h in range(QH)]

        mm_ps = root.enter_context(tc.tile_pool(name="mmps", bufs=2, space="PSUM"))
        s_ps = root.enter_context(tc.tile_pool(name="sps", bufs=3, space="PSUM"))
        y_ps = root.enter_context(tc.tile_pool(name="yps", bufs=2, space="PSUM"))
        d_ps = root.enter_context(tc.tile_pool(name="dps", bufs=1, space="PSUM"))
        sqp = root.enter_context(tc.tile_pool(name="sq", bufs=3))
        srp = root.enter_context(tc.tile_pool(name="sr", bufs=2))
        ptp = root.enter_context(tc.tile_pool(name="pt", bufs=8))
        pap = root.enter_context(tc.tile_pool(name="pa", bufs=4))
        denp = root.enter_context(tc.tile_pool(name="den", bufs=2))
        vtmp = root.enter_context(tc.tile_pool(name="vtmp", bufs=2))
        xswp = root.enter_context(tc.tile_pool(name="xswp", bufs=6))
        ropet = root.enter_context(tc.tile_pool(name="ropet", bufs=3))
        ostg = root.enter_context(tc.tile_pool(name="ost", bufs=2))

        def emit_qkv_rope(g):
            """QKV projection + norm + rope for token group g."""
            gsl = slice(g * TG, (g + 1) * TG)
            if g == 0:
                xb = x0_sb
            else:
                xb = xin.tile([P, KT, TG], bf16, tag="xb", name="xb")
                nc.sync.dma_start(out=xb[:, 0:8, :], in_=x_d[:, g, 0:8, :])
                nc.gpsimd.dma_start(out=xb[:, 8:16, :], in_=x_d[:, g, 8:16, :])
            xsws = {}

            def qkv_post(m, ps, mi):
                if m < 6:
                    # free the PSUM slot immediately: copy to SBUF first,
                    # then the whole norm chain runs off the SBUF copy, so
                    # an ACT table switch can't back up the matmul pipeline
                    nc.vector.tensor_copy(qn[m][:, gsl], ps[:])
                    # rms-norm: broadcast sum-of-squares via all-ones MM
                    sq = sqp.tile([P, TG], bf16)
                    nc.scalar.activation(sq[:], qn[m][:, gsl], AF.Square)
                    ssq = s_ps.tile([P, TG], f32, tag="s")
                    nc.tensor.matmul(ssq[:], onesm_sb[:], sq[:],
                                     start=True, stop=True)
                    srb = srp.tile([P, TG], f32)
                    nc.scalar.activation(srb[:], ssq[:], AF.Sqrt,
                                         bias=eps_sb[:], scale=1.0 / HD)
                    nc.vector.reciprocal_approx_fast(srb[:], srb[:])
                    nc.vector.tensor_mul(qn[m][:, gsl], qn[m][:, gsl], srb[:])
                    # issue the rope half-swap immediately; consumed after
                    # the m-loop.  Alternate DMA queues to avoid serializing.
                    xsw = xswp.tile([P, TG], bf16, tag="xsw")
                    eng = nc.gpsimd if mi % 2 == 0 else nc.sync
                    eng.dma_start(out=xsw[0:64, :], in_=qn[m][64:128, gsl])
                    eng.dma_start(out=xsw[64:128, :], in_=qn[m][0:64, gsl])
                    xsws[m] = xsw
                else:
                    vh = m - 6
                    vtm = vtmp.tile([P, TG], bf16)
                    nc.vector.tensor_copy(vtm[:], ps[:])
                    for jj in range(4):
                        tp = s_ps.tile([P, P], bf16, tag="s")
                        nc.tensor.transpose(
                            tp[:], vtm[:, jj * P:(jj + 1) * P], eye_sb[:])
                        nc.vector.tensor_copy(vT_sb[:, vh, 4 * g + jj], tp[:])

            if g == 0:
                # k-outer in two 4-quarter batches (4 PSUM banks each) so the
                # PE consumes wq/x chunks as the staged DMAs land
                for batch in ((0, 1, 2, 3), (4, 5, 6, 7)):
                    pss = [mm_ps.tile([P, TG], f32, tag="mm", name="ps_a"),
                           mm_ps.tile([P, TG], f32, tag="mm", name="ps_b"),
                           y_ps.tile([P, TG], f32, tag="y", name="ps_c"),
                           d_ps.tile([P, TG], f32, tag="d", name="ps_d")]
                    for k in range(KT):
                        for i, m in enumerate(batch):
                            nc.tensor.matmul(
                                pss[i][:],
                                wq_sb[:, k, m * P:(m + 1) * P],
                                xb[:, k],
                                start=(k == 0),
                                stop=(k == KT - 1),
                            )
                    for i, m in enumerate(batch):
                        qkv_post(m, pss[i], i)
            else:
                for mi, m in enumerate((4, 5, 0, 1, 2, 3, 6, 7)):
                    ps = mm_ps.tile([P, TG], f32, tag="mm")
                    for k in range(KT):
                        nc.tensor.matmul(
                            ps[:],
                            wq_sb[:, k, m * P:(m + 1) * P],
                            xb[:, k],
                            start=(k == 0),
                            stop=(k == KT - 1),
                        )
                    qkv_post(m, ps, mi)

            return xsws

        def emit_rope(g, xsws):
            """Rope for group g (k quarters first)."""
            gsl = slice(g * TG, (g + 1) * TG)
            for m in (4, 5, 0, 1, 2, 3):
                xsw = xsws[m]
                t1 = ropet.tile([P, TG], bf16, tag="t1")
                nc.vector.tensor_mul(t1[:], qn[m][:, gsl], cs_sb[:, 0, gsl])
                nc.vector.tensor_mul(xsw[:], xsw[:], cs_sb[:, 1, gsl])
                nc.vector.tensor_add(qn[m][:, gsl], t1[:], xsw[:])

        def emit_attn(g):
            """Attention for token group g."""
            gsl = slice(g * TG, (g + 1) * TG)
            jn = 4 * g + 4
            for qh in range(QH):
                kv = qh // 2
                k_t = qn[4 + kv]
                q_g = qn[qh][:, gsl]
                yp = y_ps.tile([P, TG], f32, tag="y")
                dp = d_ps.tile([P, TG], f32, tag="d")
                pts = []   # (ap, off) pending for the den chain
                for j in range(jn):
                    off = (j - 4 * g) * P if j >= 4 * g else 0
                    diag = j >= 4 * g
                    s = s_ps.tile([P, TG], f32, tag="s")
                    nc.tensor.matmul(
                        s[:, off:],
                        k_t[:, j * P:(j + 1) * P],
                        q_g[:, off:],
                        start=True,
                        stop=not diag,
                        skip_group_check=diag,
                    )
                    if diag:
                        # add -1e30 to the below-diagonal triangle on PE, so
                        # exp maps it to 0 (no cross-engine mask dependency)
                        nc.tensor.matmul(
                            s[:, off:off + P], trineg_sb[:], eye_sb[:],
                            start=False, stop=True,
                            skip_group_check=True,
                        )
                    pt = ptp.tile([P, TG], bf16)
                    nc.scalar.activation(pt[:, off:], s[:, off:], AF.Exp,
                                         scale=SCALE)
                    nc.tensor.matmul(
                        yp[:, off:], vT_sb[:, kv, j], pt[:, off:],
                        start=(j == 0), stop=(j == jn - 1),
                        skip_group_check=True,
                    )
                    # denominator: pair-add full tiles on DVE, chain on PE
                    if off == 0 and j % 2 == 0 and j + 1 < 4 * g:
                        pts.append((pt, -1))    # -1: waiting for partner
                    elif off == 0 and pts and pts[-1][1] == -1:
                        pa = pap.tile([P, TG], bf16)
                        nc.vector.tensor_add(pa[:], pts[-1][0][:], pt[:])
                        pts[-1] = (pa, 0)
                    else:
                        pts.append((pt, off))
                nd = len(pts)
                for i, (pa, off) in enumerate(pts):
                    assert off >= 0
                    nc.tensor.matmul(
                        dp[:, off:], onesm_sb[:], pa[:, off:],
                        start=(i == 0), stop=(i == nd - 1),
                        skip_group_check=True,
                    )
                den = denp.tile([P, TG], f32)
                if qh == QH - 1:
                    # chunk recip+mul per token-tile: the last head gates the
                    # out-projection, so let its first tile finish early
                    for u in range(4):
                        usl = slice(u * P, (u + 1) * P)
                        nc.vector.reciprocal_approx_fast(den[:, usl],
                                                         dp[:, usl])
                        nc.vector.tensor_mul(
                            yT[qh][:, g * TG + u * P: g * TG + (u + 1) * P],
                            yp[:, usl], den[:, usl])
                else:
                    nc.vector.reciprocal_approx_fast(den[:], dp[:])
                    nc.vector.tensor_mul(yT[qh][:, gsl], yp[:], den[:])

        def emit_outproj(g):
            """Out-projection for token group g."""
            for tt in range(4 * g, 4 * g + 4):
                ost = ostg.tile([P, C], bf16)
                for og in range(4):
                    op = mm_ps.tile([P, TG], f32, tag="mm", name="op")
                    for h in range(QH):
                        nc.tensor.matmul(
                            op[:], yT[h][:, tt * P:(tt + 1) * P],
                            wp_sb[:, h, og * TG:(og + 1) * TG],
                            start=(h == 0), stop=(h == QH - 1),
                        )
                    if og % 2 == 0:
                        nc.vector.tensor_copy(ost[:, og * TG:(og + 1) * TG], op[:])
                    else:
                        nc.scalar.copy(ost[:, og * TG:(og + 1) * TG], op[:])
                    # store each 512-col block as its copy lands, so the
                    # final DMA only waits on the last og's copy
                    eng = nc.sync if og % 2 == 0 else nc.gpsimd
                    eng.dma_start(
                        out=out_ap[tt * P:(tt + 1) * P, og * TG:(og + 1) * TG],
                        in_=ost[:, og * TG:(og + 1) * TG])

        # software pipeline: emit QKV(g+1) BEFORE attention(g) so the
        # scheduler can fill exp-gated attention bubbles with QKV matmuls;
        # attention(g) DVE ops precede rope(g+1) in the vector FIFO (rope
        # has a full iteration of slack, attention gates the out-proj)
        for i in range(G + 1):
            if i < G:
                xsws_i = emit_qkv_rope(i)
            if i >= 1:
                emit_attn(i - 1)
            if i < G:
                emit_rope(i, xsws_i)
            if i >= 1:
                emit_outproj(i - 1)


def build_nc():
    """Build and compile the (single, shared across cores) Bass program."""
    if "nc" in _CACHE:
        return _CACHE["nc"]
    import concourse.mybir as mybir
    import concourse.tile as tile
    from concourse import bacc

    bf16 = mybir.dt.bfloat16

    nc = bacc.Bacc("TRN2", target_bir_lowering=False, debug=False)
    shapes = {
        "x_sw": ((P, G, KT, TG), bf16),
        "wq_sw": ((P, KT, MQ * P), bf16),
        "wp_sw": ((P, QH, C), bf16),
        "cs_sw": ((P, 2, T), bf16),
        "trineg_sw": ((P, P), bf16),
        "eye_sw": ((P, P), bf16),
    }
    t_in = {
        name: nc.dram_tensor(name, shape, dt, kind="ExternalInput").ap()
        for name, (shape, dt) in shapes.items()
    }
    out_ap = nc.dram_tensor("out", (T, C), bf16, kind="ExternalOutput").ap()

    with tile.TileContext(nc) as tc:
        _emit(tc, out_ap, t_in)
    nc.compile()
    _CACHE["nc"] = nc
    return nc


# --------------------------------------------------------------------------
# host-side data preparation
# --------------------------------------------------------------------------

def _swizzle_ktiles(a2d):
    """[R*128, F] -> [128, R, F] picking partition-within-tile as leading."""
    r128, f = a2d.shape
    r = r128 // P
    return np.ascontiguousarray(a2d.reshape(r, P, f).transpose(1, 0, 2))


def host_prep(x, w_attn, w_proj, cos, sin):
    x = np.asarray(x, np.float32)
    w_attn = np.asarray(w_attn, np.float32)
    w_proj = np.asarray(w_proj, np.float32)
    cos = np.asarray(cos, np.float32).reshape(T, HD // 2)
    sin = np.asarray(sin, np.float32).reshape(T, HD // 2)

    # x per batch: (T, C) -> [128, g, k, t]
    x_sws = []
    for b in range(B):
        xb = x[b].reshape(G, TG, KT, P).transpose(3, 0, 2, 1)
        x_sws.append(np.ascontiguousarray(xb).astype(BF16))

    # cos/sin duplicated across both 64-partition halves: [128, 2, T]
    c2 = np.concatenate([cos.T, cos.T], axis=0)     # (128, T)
    s2 = np.concatenate([sin.T, -sin.T], axis=0)    # sign-folded for rope add
    cs_sw = np.stack([c2, s2], axis=1).astype(BF16)  # (128, 2, T)

    col = np.arange(P)[None, :]
    row = np.arange(P)[:, None]
    # M[r,c] = 0 where causal-live (c >= r), -1e30 where masked; the device
    # adds M to the diagonal score block via lhsT = M.T (out[i,j] = lhsT[j,i])
    m_mask = np.where(col >= row, 0.0, -1e30).astype(np.float32)
    trineg_sw = np.ascontiguousarray(m_mask.T).astype(BF16)
    eye_sw = np.eye(P, dtype=np.float32).astype(BF16)

    in_maps = []
    for c in range(N_CORES):
        b, q = divmod(c, 4)
        qrows = w_attn[QH * HD * q: QH * HD * (q + 1)]
        krows = w_attn[C + KVH * HD * q: C + KVH * HD * (q + 1)]
        vrows = w_attn[C + KV_DIM + KVH * HD * q: C + KV_DIM + KVH * HD * (q + 1)]
        w_sel = np.concatenate([qrows, krows, vrows], axis=0)   # (1024, C)
        wq_sw = _swizzle_ktiles(w_sel.T).astype(BF16)           # (128, 16, 1024)

        wp_sel = w_proj[:, QH * HD * q: QH * HD * (q + 1)]      # (C, 512)
        wp_sw = _swizzle_ktiles(np.ascontiguousarray(wp_sel.T)).astype(BF16)

        in_maps.append({
            "x_sw": x_sws[b],
            "wq_sw": np.ascontiguousarray(wq_sw),
            "wp_sw": np.ascontiguousarray(wp_sw),   # (128, 4, 2048)
            "cs_sw": cs_sw,
            "trineg_sw": trineg_sw,
            "eye_sw": eye_sw,
        })
    return in_maps


def run_on_hw(in_maps, trace=False, **kwargs):
    from concourse import bass_utils

    nc = build_nc()
    return bass_utils.run_bass_kernel_spmd(
        nc, in_maps, core_ids=list(range(N_CORES)), trace=trace, **kwargs
    )


def gather(res):
    """Sum the 4 partial outputs per batch -> (B, T, C) float32."""
    out = np.zeros((B, T, C), np.float32)
    for c, r in enumerate(res.results):
        out[c // 4] += r["out"].astype(np.float32)
    return out


def kernel(x, w_attn, w_proj, cos, sin):
    in_maps = host_prep(x, w_attn, w_proj, cos, sin)
    res = run_on_hw(in_maps)
    return gather(res)



# revision 17
# speedup vs baseline: 1.0789x; 1.0408x over previous
"""Trainium2 Bass kernel for nn_CausalSelfAttention (GQA + RoPE + qk-RMSNorm).

Sharding: batch x head-quad over 8 NeuronCores.
  - Core c: batch = c // 4, quad = c % 4.
  - Each core owns 4 of the 16 q heads (4*quad .. 4*quad+3) and the matching
    2 of 8 kv heads (2*quad, 2*quad+1) for ONE batch element.
  - Per core: QKV projection for its 1024 rows of w_attn over its batch's
    2048 tokens, RoPE + qk RMS norm, causal attention, partial output
    projection through its 512 columns of w_proj.
  - Host sums the 4 partial outputs per batch (no on-device collectives).

Fused per-token-group pipeline: for each 512-token group g we run
QKV -> rope/norm -> attention (flash-style, causal-sliced) -> out-proj, so
the tensor engine always has dense matmul work while exp/softmax runs on
the scalar/vector engines.

Matmuls run in bf16 with fp32 PSUM accumulation; softmax/statistics fp32.
Self-contained: hardcodes all shapes from the problem spec.
"""

import math
import numpy as np
import ml_dtypes
from contextlib import ExitStack

# ---- problem constants (hardcoded per spec) ----
B, T, C = 2, 2048, 2048
N_HEAD, N_KV_HEAD, HD = 16, 8, 128
KV_DIM = N_KV_HEAD * HD
EPS = 1.1920929e-07
N_CORES = 8
P = 128
TG = 512                                 # token group (matmul N)
G = T // TG                              # 4 token groups per core
KT = C // P                              # 16 contraction tiles
QH = 4                                   # q heads per core
KVH = 2                                  # kv heads per core
MQ = QH + 2 * KVH                        # 8 row-quarters of the 1024 QKV rows
NJ = T // P                              # 16 k tiles
SCALE = 1.0 / math.sqrt(HD)

BF16 = ml_dtypes.bfloat16

_CACHE = {}


# --------------------------------------------------------------------------
# device program
# --------------------------------------------------------------------------

def _emit(tc, out_ap, t_in):
    import concourse.bass as bass  # noqa: F401
    import concourse.mybir as mybir

    f32 = mybir.dt.float32
    bf16 = mybir.dt.bfloat16
    AF = mybir.ActivationFunctionType
    nc = tc.nc

    x_d = t_in["x_sw"]
    wq_d = t_in["wq_sw"]
    wp_d = t_in["wp_sw"]
    cs_d = t_in["cs_sw"]
    trineg_d = t_in["trineg_sw"]
    eye_d = t_in["eye_sw"]

    with ExitStack() as root:
        const = root.enter_context(tc.tile_pool(name="const", bufs=1))
        xin = root.enter_context(tc.tile_pool(name="xin", bufs=2))
        # fine-grained interleaved staging: QKV(g=0) runs k-outer over the
        # q-head half of wq, so chunk k-tiles of wq/x land just ahead of use.
        # wq on the sync queue, x on the gpsimd queue -- parallel streams.
        wq_sb = const.tile([P, KT, MQ * P], bf16)
        x0_sb = xin.tile([P, KT, TG], bf16, tag="xb")
        for k0 in range(0, KT, 2):
            nc.sync.dma_start(out=wq_sb[:, k0:k0 + 2, 0:4 * P],
                              in_=wq_d[:, k0:k0 + 2, 0:4 * P])
            nc.sync.dma_start(out=x0_sb[:, k0:k0 + 2, :],
                              in_=x_d[:, 0, k0:k0 + 2, :])
        for k0 in range(0, KT, 4):
            nc.sync.dma_start(out=wq_sb[:, k0:k0 + 4, 4 * P:],
                              in_=wq_d[:, k0:k0 + 4, 4 * P:])
        eye_sb = const.tile([P, P], bf16)
        nc.sync.dma_start(out=eye_sb[:], in_=eye_d)
        cs_sb = const.tile([P, 2, T], bf16)
        nc.sync.dma_start(out=cs_sb[:], in_=cs_d)
        trineg_sb = const.tile([P, P], bf16)
        nc.sync.dma_start(out=trineg_sb[:], in_=trineg_d)
        wp_sb = const.tile([P, QH, C], bf16)
        nc.sync.dma_start(out=wp_sb[:], in_=wp_d)
        eps_sb = const.tile([P, 1], f32)
        nc.vector.memset(eps_sb[:], EPS)
        onesm_sb = const.tile([P, P], bf16)
        nc.vector.memset(onesm_sb[:], 1.0)

        big = root.enter_context(tc.tile_pool(name="big", bufs=1))
        # post-rope, post-norm q (4 heads) + k (2 heads), [d, tok] layout
        qn = [big.tile([P, T], bf16, name=f"qn{m}", tag=f"qn{m}")
              for m in range(6)]
        vT_sb = big.tile([P, KVH, NJ, P], bf16, tag="vT")  # [ktok, vh, j, d]
        yT = [big.tile([P, T], bf16, name=f"yT{h}", tag=f"yT{h}")
              for h in range(QH)]

        mm_ps = root.enter_context(tc.tile_pool(name="mmps", bufs=2, space="PSUM"))
        s_ps = root.enter_context(tc.tile_pool(name="sps", bufs=3, space="PSUM"))
        y_ps = root.enter_context(tc.tile_pool(name="yps", bufs=2, space="PSUM"))
        d_ps = root.enter_context(tc.tile_pool(name="dps", bufs=1, space="PSUM"))
        sqp = root.enter_context(tc.tile_pool(name="sq", bufs=3))
        srp = root.enter_context(tc.tile_pool(name="sr", bufs=2))
        ptp = root.enter_context(tc.tile_pool(name="pt", bufs=8))
        pap = root.enter_context(tc.tile_pool(name="pa", bufs=4))
        denp = root.enter_context(tc.tile_pool(name="den", bufs=2))
        vtmp = root.enter_context(tc.tile_pool(name="vtmp", bufs=2))
        xswp = root.enter_context(tc.tile_pool(name="xswp", bufs=6))
        ropet = root.enter_context(tc.tile_pool(name="ropet", bufs=3))
        ostg = root.enter_context(tc.tile_pool(name="ost", bufs=2))

        def emit_qkv_rope(g):
            """QKV projection + norm + rope for token group g."""
            gsl = slice(g * TG, (g + 1) * TG)
            if g == 0:
                xb = x0_sb
            else:
                xb = xin.tile([P, KT, TG], bf16, tag="xb", name="xb")
                nc.sync.dma_start(out=xb[:, 0:8, :], in_=x_d[:, g, 0:8, :])
                nc.sync.dma_start(out=xb[:, 8:16, :], in_=x_d[:, g, 8:16, :])
            xsws = {}

            def qkv_post(m, ps, mi):
                if m < 6:
                    # free the PSUM slot immediately: copy to SBUF first,
                    # then the whole norm chain runs off the SBUF copy, so
                    # an ACT table switch can't back up the matmul pipeline
                    nc.vector.tensor_copy(qn[m][:, gsl], ps[:])
                    # rms-norm: broadcast sum-of-squares via all-ones MM
                    sq = sqp.tile([P, TG], bf16)
                    nc.scalar.activation(sq[:], qn[m][:, gsl], AF.Square)
                    ssq = s_ps.tile([P, TG], f32, tag="s")
                    nc.tensor.matmul(ssq[:], onesm_sb[:], sq[:],
                                     start=True, stop=True)
                    srb = srp.tile([P, TG], f32)
                    nc.scalar.activation(srb[:], ssq[:], AF.Sqrt,
                                         bias=eps_sb[:], scale=1.0 / HD)
                    nc.vector.reciprocal_approx_fast(srb[:], srb[:])
                    nc.vector.tensor_mul(qn[m][:, gsl], qn[m][:, gsl], srb[:])
                    # issue the rope half-swap immediately; consumed after
                    # the m-loop.  Alternate DMA queues to avoid serializing.
                    xsw = xswp.tile([P, TG], bf16, tag="xsw")
                    eng = nc.gpsimd if mi % 2 == 0 else nc.sync
                    eng.dma_start(out=xsw[0:64, :], in_=qn[m][64:128, gsl])
                    eng.dma_start(out=xsw[64:128, :], in_=qn[m][0:64, gsl])
                    xsws[m] = xsw
                else:
                    vh = m - 6
                    vtm = vtmp.tile([P, TG], bf16)
                    nc.vector.tensor_copy(vtm[:], ps[:])
                    for jj in range(4):
                        tp = s_ps.tile([P, P], bf16, tag="s")
                        nc.tensor.transpose(
                            tp[:], vtm[:, jj * P:(jj + 1) * P], eye_sb[:])
                        nc.vector.tensor_copy(vT_sb[:, vh, 4 * g + jj], tp[:])

            if g == 0:
                # k-outer in two 4-quarter batches (4 PSUM banks each) so the
                # PE consumes wq/x chunks as the staged DMAs land
                for batch in ((0, 1, 2, 3), (4, 5, 6, 7)):
                    pss = [mm_ps.tile([P, TG], f32, tag="mm", name="ps_a"),
                           mm_ps.tile([P, TG], f32, tag="mm", name="ps_b"),
                           y_ps.tile([P, TG], f32, tag="y", name="ps_c"),
                           d_ps.tile([P, TG], f32, tag="d", name="ps_d")]
                    for k in range(KT):
                        for i, m in enumerate(batch):
                            nc.tensor.matmul(
                                pss[i][:],
                                wq_sb[:, k, m * P:(m + 1) * P],
                                xb[:, k],
                                start=(k == 0),
                                stop=(k == KT - 1),
                            )
                    for i, m in enumerate(batch):
                        qkv_post(m, pss[i], i)
            else:
                for mi, m in enumerate((4, 5, 0, 1, 2, 3, 6, 7)):
                    ps = mm_ps.tile([P, TG], f32, tag="mm")
                    for k in range(KT):
                        nc.tensor.matmul(
                            ps[:],
                            wq_sb[:, k, m * P:(m + 1) * P],
                            xb[:, k],
                            start=(k == 0),
                            stop=(k == KT - 1),
                        )
                    qkv_post(m, ps, mi)

            return xsws

        def emit_rope(g, xsws):
            """Rope for group g (k quarters first)."""
            gsl = slice(g * TG, (g + 1) * TG)
            for m in (4, 5, 0, 1, 2, 3):
                xsw = xsws[m]
                t1 = ropet.tile([P, TG], bf16, tag="t1")
                nc.vector.tensor_mul(t1[:], qn[m][:, gsl], cs_sb[:, 0, gsl])
                nc.vector.tensor_mul(xsw[:], xsw[:], cs_sb[:, 1, gsl])
                nc.vector.tensor_add(qn[m][:, gsl], t1[:], xsw[:])

        def emit_attn(g):
            """Attention for token group g."""
            gsl = slice(g * TG, (g + 1) * TG)
            jn = 4 * g + 4
            for qh in range(QH):
                kv = qh // 2
                k_t = qn[4 + kv]
                q_g = qn[qh][:, gsl]
                yp = y_ps.tile([P, TG], f32, tag="y")
                dp = d_ps.tile([P, TG], f32, tag="d")
                pts = []   # (ap, off) pending for the den chain
                for j in range(jn):
                    off = (j - 4 * g) * P if j >= 4 * g else 0
                    diag = j >= 4 * g
                    s = s_ps.tile([P, TG], f32, tag="s")
                    nc.tensor.matmul(
                        s[:, off:],
                        k_t[:, j * P:(j + 1) * P],
                        q_g[:, off:],
                        start=True,
                        stop=not diag,
                        skip_group_check=diag,
                    )
                    if diag:
                        # add -1e30 to the below-diagonal triangle on PE, so
                        # exp maps it to 0 (no cross-engine mask dependency)
                        nc.tensor.matmul(
                            s[:, off:off + P], trineg_sb[:], eye_sb[:],
                            start=False, stop=True,
                            skip_group_check=True,
                        )
                    pt = ptp.tile([P, TG], bf16)
                    nc.scalar.activation(pt[:, off:], s[:, off:], AF.Exp,
                                         scale=SCALE)
                    nc.tensor.matmul(
                        yp[:, off:], vT_sb[:, kv, j], pt[:, off:],
                        start=(j == 0), stop=(j == jn - 1),
                        skip_group_check=True,
                    )
                    # denominator: pair-add full tiles on DVE, chain on PE
                    if off == 0 and j % 2 == 0 and j + 1 < 4 * g:
                        pts.append((pt, -1))    # -1: waiting for partner
                    elif off == 0 and pts and pts[-1][1] == -1:
                        pa = pap.tile([P, TG], bf16)
                        nc.vector.tensor_add(pa[:], pts[-1][0][:], pt[:])
                        pts[-1] = (pa, 0)
                    else:
                        pts.append((pt, off))
                nd = len(pts)
                for i, (pa, off) in enumerate(pts):
                    assert off >= 0
                    nc.tensor.matmul(
                        dp[:, off:], onesm_sb[:], pa[:, off:],
                        start=(i == 0), stop=(i == nd - 1),
                        skip_group_check=True,
                    )
                den = denp.tile([P, TG], f32)
                if qh == QH - 1:
                    # chunk recip+mul per token-tile: the last head gates the
                    # out-projection, so let its first tile finish early
                    for u in range(4):
                        usl = slice(u * P, (u + 1) * P)
                        nc.vector.reciprocal_approx_fast(den[:, usl],
                                                         dp[:, usl])
                        nc.vector.tensor_mul(
                            yT[qh][:, g * TG + u * P: g * TG + (u + 1) * P],
                            yp[:, usl], den[:, usl])
                else:
                    nc.vector.reciprocal_approx_fast(den[:], dp[:])
                    nc.vector.tensor_mul(yT[qh][:, gsl], yp[:], den[:])

        def emit_outproj(g):
            """Out-projection for token group g."""
            for tt in range(4 * g, 4 * g + 4):
                ost = ostg.tile([P, C], bf16)
                for og in range(4):
                    op = mm_ps.tile([P, TG], f32, tag="mm", name="op")
                    for h in range(QH):
                        nc.tensor.matmul(
                            op[:], yT[h][:, tt * P:(tt + 1) * P],
                            wp_sb[:, h, og * TG:(og + 1) * TG],
                            start=(h == 0), stop=(h == QH - 1),
                        )
                    if og % 2 == 0:
                        nc.vector.tensor_copy(ost[:, og * TG:(og + 1) * TG], op[:])
                    else:
                        nc.scalar.copy(ost[:, og * TG:(og + 1) * TG], op[:])
                    # store each 512-col block as its copy lands, so the
                    # final DMA only waits on the last og's copy
                    nc.sync.dma_start(
                        out=out_ap[tt * P:(tt + 1) * P, og * TG:(og + 1) * TG],
                        in_=ost[:, og * TG:(og + 1) * TG])

        # software pipeline: emit QKV(g+1) BEFORE attention(g) so the
        # scheduler can fill exp-gated attention bubbles with QKV matmuls;
        # attention(g) DVE ops precede rope(g+1) in the vector FIFO (rope
        # has a full iteration of slack, attention gates the out-proj)
        for i in range(G + 1):
            if i < G:
                xsws_i = emit_qkv_rope(i)
            if i >= 1:
                emit_attn(i - 1)
            if i < G:
                emit_rope(i, xsws_i)
            if i >= 1:
                emit_outproj(i - 1)


def build_nc():
    """Build and compile the (single, shared across cores) Bass program."""
    if "nc" in _CACHE:
        return _CACHE["nc"]
    import concourse.mybir as mybir
    import concourse.tile as tile
    from concourse import bacc

    bf16 = mybir.dt.bfloat16

    nc = bacc.Bacc("TRN2", target_bir_lowering=False, debug=False)
    shapes = {
        "x_sw": ((P, G, KT, TG), bf16),
        "wq_sw": ((P, KT, MQ * P), bf16),
        "wp_sw": ((P, QH, C), bf16),
        "cs_sw": ((P, 2, T), bf16),
        "trineg_sw": ((P, P), bf16),
        "eye_sw": ((P, P), bf16),
    }
    t_in = {
        name: nc.dram_tensor(name, shape, dt, kind="ExternalInput").ap()
        for name, (shape, dt) in shapes.items()
    }
    out_ap = nc.dram_tensor("out", (T, C), bf16, kind="ExternalOutput").ap()

    with tile.TileContext(nc) as tc:
        _emit(tc, out_ap, t_in)
    nc.compile()
    _CACHE["nc"] = nc
    return nc


# --------------------------------------------------------------------------
# host-side data preparation
# --------------------------------------------------------------------------

def _swizzle_ktiles(a2d):
    """[R*128, F] -> [128, R, F] picking partition-within-tile as leading."""
    r128, f = a2d.shape
    r = r128 // P
    return np.ascontiguousarray(a2d.reshape(r, P, f).transpose(1, 0, 2))


def host_prep(x, w_attn, w_proj, cos, sin):
    x = np.asarray(x, np.float32)
    w_attn = np.asarray(w_attn, np.float32)
    w_proj = np.asarray(w_proj, np.float32)
    cos = np.asarray(cos, np.float32).reshape(T, HD // 2)
    sin = np.asarray(sin, np.float32).reshape(T, HD // 2)

    # x per batch: (T, C) -> [128, g, k, t]
    x_sws = []
    for b in range(B):
        xb = x[b].reshape(G, TG, KT, P).transpose(3, 0, 2, 1)
        x_sws.append(np.ascontiguousarray(xb).astype(BF16))

    # cos/sin duplicated across both 64-partition halves: [128, 2, T]
    c2 = np.concatenate([cos.T, cos.T], axis=0)     # (128, T)
    s2 = np.concatenate([sin.T, -sin.T], axis=0)    # sign-folded for rope add
    cs_sw = np.stack([c2, s2], axis=1).astype(BF16)  # (128, 2, T)

    col = np.arange(P)[None, :]
    row = np.arange(P)[:, None]
    # M[r,c] = 0 where causal-live (c >= r), -1e30 where masked; the device
    # adds M to the diagonal score block via lhsT = M.T (out[i,j] = lhsT[j,i])
    m_mask = np.where(col >= row, 0.0, -1e30).astype(np.float32)
    trineg_sw = np.ascontiguousarray(m_mask.T).astype(BF16)
    eye_sw = np.eye(P, dtype=np.float32).astype(BF16)

    in_maps = []
    for c in range(N_CORES):
        b, q = divmod(c, 4)
        qrows = w_attn[QH * HD * q: QH * HD * (q + 1)]
        krows = w_attn[C + KVH * HD * q: C + KVH * HD * (q + 1)]
        vrows = w_attn[C + KV_DIM + KVH * HD * q: C + KV_DIM + KVH * HD * (q + 1)]
        w_sel = np.concatenate([qrows, krows, vrows], axis=0)   # (1024, C)
        wq_sw = _swizzle_ktiles(w_sel.T).astype(BF16)           # (128, 16, 1024)

        wp_sel = w_proj[:, QH * HD * q: QH * HD * (q + 1)]      # (C, 512)
        wp_sw = _swizzle_ktiles(np.ascontiguousarray(wp_sel.T)).astype(BF16)

        in_maps.append({
            "x_sw": x_sws[b],
            "wq_sw": np.ascontiguousarray(wq_sw),
            "wp_sw": np.ascontiguousarray(wp_sw),   # (128, 4, 2048)
            "cs_sw": cs_sw,
            "trineg_sw": trineg_sw,
            "eye_sw": eye_sw,
        })
    return in_maps


def run_on_hw(in_maps, trace=False, **kwargs):
    from concourse import bass_utils

    nc = build_nc()
    return bass_utils.run_bass_kernel_spmd(
        nc, in_maps, core_ids=list(range(N_CORES)), trace=trace, **kwargs
    )


def gather(res):
    """Sum the 4 partial outputs per batch -> (B, T, C) float32."""
    out = np.zeros((B, T, C), np.float32)
    for c, r in enumerate(res.results):
        out[c // 4] += r["out"].astype(np.float32)
    return out


def kernel(x, w_attn, w_proj, cos, sin):
    in_maps = host_prep(x, w_attn, w_proj, cos, sin)
    res = run_on_hw(in_maps)
    return gather(res)

